# revision 6
# baseline (speedup 1.0000x reference)
"""Bottom-Up Hidden Tree Markov Model upward pass on 8 Trainium2 NeuronCores.

Problem: complete 8-ary forest (2 trees x 299593 nodes, depth 6), C=8 hidden
states, 32 symbols, 16 independent generative models. Output: per-tree
log-likelihood (2, 16).

Sharding: core = (tree, quarter-of-tree). Each core runs the upward pass over
its quarter's two big levels (98.4% of all nodes): 65536 leaves -> 8192
level-5 betas (one fp8 DoubleRow matmul per 512 parents against the collapsed
leaf table T6, contracting all 256 (position, symbol) one-hot rows at once)
and 8192 -> 1024 level-4 t_betas (block-diagonal W matmuls). The host applies
the exact softmax recursion to the tiny tree top (levels 4..1 + root, 1.6% of
nodes) and the leaf log-nu histogram, both in float64.

Device pipeline per 512-parent chunk (engines balanced, dispatch-minimal):
  PE:   tb   = [T6a;T6b]^T [ohA;ohB]           (fp8 DoubleRow, PSUM f32)
  DVE:  bl   = tb * bx                          (bf16 out)
  PE:   nu_b = SR^T bl                          (per-g sums broadcast to all
                                                 128 partitions in one matmul)
  Act:  Ln(nu) with free-dim accumulation       -> per-chunk log-nu partials
  Pool: beta = bl / nu_b                        (divide, bf16 out)
Partition packing everywhere: p = i*16 + g  (i = hidden state, g = generator).

T6 and bx are carried in fp8_e4m3 scaled by 128 (one-hots exact, ~2^-4
relative quantization); the uniform 128^2 scale cancels in beta and is
subtracted from the accumulated log-nus on the host.
"""
import sys

import numpy as np

if '/opt/trn_rl_repo' not in sys.path:
    sys.path.insert(0, '/opt/trn_rl_repo')

import ml_dtypes

BF16 = ml_dtypes.bfloat16
F8 = ml_dtypes.float8_e4m3

K, DEPTH, NTREE, C, MSYM, NGEN = 8, 6, 2, 8, 32, 16
STARTS = [(K ** d - 1) // (K - 1) for d in range(DEPTH + 2)]
NT = STARTS[DEPTH + 1]          # 299593 nodes per tree
CG = C * NGEN                   # 128
NQ = 4                          # quarters per tree (core = tree x quarter)
LEAVES_Q = (K ** DEPTH) // NQ   # 65536 leaves per core
NP5 = LEAVES_Q // K             # 8192 level-5 parents per core
NP4 = NP5 // K                  # 1024 level-4 parents per core
CW = 512                        # chunk width (one PSUM bank of f32)
NCH = NP5 // CW                 # 16 chunks
SCALE = 128.0                   # fp8 range centering; cancels in beta
_SR_OFF = 256                   # byte offsets in the small packed table
TABB = 512


def _softmax64(x, axis):
    x = np.asarray(x, np.float64)
    e = np.exp(x - x.max(axis=axis, keepdims=True))
    return e / e.sum(axis=axis, keepdims=True)


def _build_tables(A, B, Pi, SP):
    """Small O(params) tables, f64 on host."""
    smA = _softmax64(A, 0)            # (C,C,K,G) over parent state i
    smB = _softmax64(B, 1)            # (C,M,G) over symbols
    smPi = _softmax64(Pi, 0)          # (C,K,G)
    smSP = _softmax64(SP, 0)          # (K,G)
    Mmat = smSP[:, None, None, :] * np.transpose(smA, (2, 0, 1, 3))  # [l,i,j,g]
    pb = smPi[:, :, None, :] * smB[:, None, :, :]      # (j, l, s, g)
    nuL = pb.sum(0)                                     # (l, s, g)
    betaLeaf = pb / nuL[None]
    llLeaf = np.log(nuL)                                # (l, s, g)
    T6 = np.einsum('lijg,jlsg->lsig', Mmat, betaLeaf)   # (l,s,i,g)
    T6f = (T6 * SCALE).reshape(K * MSYM, CG)            # rows (l,s), cols (i,g)
    Wl = np.zeros((K, CG, CG))
    ii = np.arange(C)
    for l in range(K):
        for g in range(NGEN):
            Wl[l, ii[:, None] * NGEN + g, ii[None, :] * NGEN + g] = Mmat[l, :, :, g].T
    Wt = np.concatenate([Wl[l] for l in range(K)], axis=1)   # [128, 1024]
    p = np.arange(CG)
    SR = (p[:, None] % NGEN == p[None, :] % NGEN).astype(np.float64)  # [128,128]
    BTcg = np.transpose(smB, (1, 0, 2)).reshape(MSYM, CG).T  # [(i,g), s]

    tabs = np.zeros((CG, TABB), np.uint8)
    tabs[:, 0:128] = T6f[:128].T.astype(F8).view(np.uint8)
    tabs[:, 128:256] = T6f[128:].T.astype(F8).view(np.uint8)
    tabs[:, _SR_OFF:_SR_OFF + 256] = SR.astype(BF16).view(np.uint8)
    return tabs, Wt.astype(BF16), Mmat, smB, BTcg, llLeaf


def _build_bass(repeat=1):
    import concourse.bass as bass
    import concourse.bacc as bacc
    import concourse.mybir as mybir
    from concourse import tile

    f32 = mybir.dt.float32
    bf16 = mybir.dt.bfloat16
    f8 = mybir.dt.float8e4
    u8 = mybir.dt.uint8
    Alu = mybir.AluOpType
    Act = mybir.ActivationFunctionType
    DR = mybir.MatmulPerfMode.DoubleRow

    nc = bacc.Bacc(None, target_bir_lowering=False)

    tabs_d = nc.dram_tensor('tabs', [CG, TABB], u8, kind='ExternalInput')
    wt_d = nc.dram_tensor('wt', [CG, 1024], bf16, kind='ExternalInput')
    oh_d = [nc.dram_tensor(f'oh{k}', [CG, 4096], f8, kind='ExternalInput')
            for k in range(4)]
    bx_d = [nc.dram_tensor(f'bx{k}', [CG, 2048], f8, kind='ExternalInput')
            for k in range(4)]
    tb4_d = [nc.dram_tensor(f'tb4_{h}', [CG, 256], f32, kind='ExternalOutput')
             for h in range(4)]
    nu5_d = nc.dram_tensor('nu5', [NGEN, NP5], f32, kind='ExternalOutput')

    with tile.TileContext(nc) as tc:
        with (
            tc.tile_pool(name='const', bufs=2) as constp,
            tc.tile_pool(name='oh', bufs=2) as ohp,
            tc.tile_pool(name='bx', bufs=2) as bxp,
            tc.tile_pool(name='bl', bufs=6) as blp,
            tc.tile_pool(name='beta', bufs=2) as betap,
            tc.tile_pool(name='acc', bufs=2) as accp,
            tc.tile_pool(name='ps_tb', bufs=3, space='PSUM') as ps_tb,
            tc.tile_pool(name='ps_nu', bufs=3, space='PSUM') as ps_nu,
            tc.tile_pool(name='ps_w', bufs=1, space='PSUM') as ps_w,
        ):
            for rep in range(repeat):
                r = f'_{rep}' if repeat > 1 else ''
                tabs_t = constp.tile([CG, TABB], u8, name=f'tabs{r}', tag='tabs')
                wt_t = constp.tile([CG, 1024], bf16, name=f'wt{r}', tag='wt')
                oh_t = [ohp.tile([CG, 4096], f8, name=f'oh{k}{r}', tag=f'oh{k}')
                        for k in range(4)]
                bx_t = bxp.tile([CG, NP5], f8, name=f'bx{r}', tag='bx')
                beta5 = betap.tile([CG, NP5], bf16, name=f'b5{r}', tag='b5')
                nu5_sb = accp.tile([NGEN, NP5], f32, name=f'nu5{r}', tag='nu5')
                tb4_sb = accp.tile([CG, NP4], f32, name=f'tb4{r}', tag='tb4')

                # input DMAs in first-need order; Pool issues via SWDGE which
                # bypasses the serial HWDGE dispatch stage, so early tensors
                # go there to cut the startup trickle
                nc.sync.dma_start(tabs_t[:], tabs_d[:])
                nc.scalar.dma_start(oh_t[0][:], oh_d[0][:])
                nc.gpsimd.dma_start(bx_t[:, 0:2048], bx_d[0][:])
                nc.gpsimd.dma_start(oh_t[1][:], oh_d[1][:])
                nc.scalar.dma_start(bx_t[:, 2048:4096], bx_d[1][:])
                nc.sync.dma_start(oh_t[2][:], oh_d[2][:])
                nc.gpsimd.dma_start(bx_t[:, 4096:6144], bx_d[2][:])
                nc.sync.dma_start(oh_t[3][:], oh_d[3][:])
                nc.scalar.dma_start(bx_t[:, 6144:NP5], bx_d[3][:])
                nc.sync.dma_start(wt_t[:], wt_d[:])

                # fp8 DoubleRow weights: k-tile 0 = T6a, k-tile 1 = T6b
                t6ab = tabs_t[:, 0:256].bitcast(f8).rearrange(
                    'p (k q) -> p k q', k=2)
                SR = tabs_t[:, _SR_OFF:_SR_OFF + 256].bitcast(bf16)

                bview = beta5[:].rearrange('p (u l) -> p u l', l=K)
                bl_tiles = {}

                def emit_head(c):
                    k, j = c // 4, c % 4
                    tb_ps = ps_tb.tile([CG, CW], f32, name=f'tb{r}', tag='tb')
                    t = oh_t[k][:]
                    rhs = bass.AP(t.tensor, t.offset + j * CW,
                                  [t.ap[0], [2048, 2], [1, CW]])
                    nc.tensor.matmul(tb_ps[:], t6ab, rhs, start=True, stop=True,
                                     perf_mode=DR)
                    bl_t = blp.tile([CG, CW], bf16, name=f'bl{r}', tag='bl')
                    nc.vector.tensor_mul(bl_t[:], tb_ps[:],
                                         bx_t[:, c * CW:(c + 1) * CW])
                    bl_tiles[c] = bl_t

                def emit_tail(c):
                    bl_t = bl_tiles.pop(c)
                    nu_ps = ps_nu.tile([CG, CW], f32, name=f'nu{r}', tag='nu')
                    nc.tensor.matmul(nu_ps[:], SR, bl_t[:], start=True, stop=True)
                    nc.scalar.copy(nu5_sb[:, c * CW:(c + 1) * CW],
                                   nu_ps[0:NGEN, :])
                    # GPSIMD/Pool cannot touch PSUM and DVE has no divide ISA
                    # op, so renorm = DVE reciprocal (PSUM -> SBUF bf16, the
                    # SR matmul already broadcast nu to all 128 partitions)
                    # followed by an all-SBUF multiply on Pool
                    r_t = blp.tile([CG, CW], bf16, name=f'rc{r}', tag='rc')
                    with nc.allow_low_precision(reason='bf16 recip, validated vs f64 host'):
                        nc.vector.reciprocal(r_t[:], nu_ps[:])
                    # Pool handles beta at steady state; DVE (idle after the
                    # last reciprocal) takes the final chunk to shorten the tail
                    mul_eng = nc.vector if c == NCH - 1 else nc.gpsimd
                    mul_eng.tensor_mul(beta5[:, c * CW:(c + 1) * CW],
                                       bl_t[:], r_t[:])

                def emit_w(ph):
                    # level 5 -> 4 over parents [128*ph, 128*(ph+1)), i.e.
                    # beta5 chunks 2ph..2ph+1; drain on DVE; ship per quarter
                    w_ps = ps_w.tile([CG, 128], f32, name=f'w{ph}{r}',
                                     tag=f'w{ph % 2}')
                    for l in range(K):
                        nc.tensor.matmul(w_ps[:], wt_t[:, 128 * l:128 * (l + 1)],
                                         bview[:, ph * 128:(ph + 1) * 128, l],
                                         start=(l == 0), stop=(l == K - 1))
                    nc.scalar.copy(tb4_sb[:, ph * 128:(ph + 1) * 128], w_ps[:])
                    if ph % 2 == 1:
                        qd = ph // 2
                        nc.scalar.dma_start(
                            tb4_d[qd][:], tb4_sb[:, qd * 256:(qd + 1) * 256])

                # software-pipelined level 6 -> 5 (tail lags head by LAG);
                # W phase ph consumes chunks 2ph..2ph+1
                LAG = 2
                for c in range(NCH):
                    emit_head(c)
                    if c >= LAG:
                        emit_tail(c - LAG)
                        if (c - LAG) % 2 == 1:
                            emit_w((c - LAG) // 2)
                for c in range(NCH - LAG, NCH):
                    emit_tail(c)
                    if c % 2 == 1:
                        emit_w(c // 2)

                nc.sync.dma_start(nu5_d[:], nu5_sb[:])
    if not nc.is_finalized():
        nc.finalize()
    return nc


_BASS_CACHE = {}


def _get_bass(repeat=1):
    if repeat not in _BASS_CACHE:
        _BASS_CACHE[repeat] = _build_bass(repeat)
    return _BASS_CACHE[repeat]


def kernel(**inputs):
    from concourse.bass_utils import run_bass_kernel_spmd

    A = np.asarray(inputs['A']); B = np.asarray(inputs['B'])
    Pi = np.asarray(inputs['Pi']); SP = np.asarray(inputs['SP'])
    x = np.asarray(inputs['x'])

    tabs, Wt_bf, Mmat, smB, BTcg, llLeaf = _build_tables(A, B, Pi, SP)
    BT_f8 = (BTcg * SCALE).astype(F8)

    in_maps = []
    for t in range(NTREE):
        base = t * NT
        for q in range(NQ):
            s6 = base + STARTS[6] + q * LEAVES_Q
            xs_t = x[s6: s6 + LEAVES_Q].reshape(NP5, K).T      # [8, 8192]
            s5 = base + STARTS[5] + q * NP5
            x5 = x[s5: s5 + NP5]
            oh = np.zeros((CG, 2 * NP5), F8)
            cols = np.arange(NP5)
            one = F8(1.0)
            for l in range(4):
                oh[l * MSYM + xs_t[l], cols] = one
                oh[l * MSYM + xs_t[4 + l], NP5 + cols] = one
            bx5 = BT_f8[:, x5]                                  # [128, 8192]
            m = {'tabs': tabs, 'wt': Wt_bf}
            for k in range(4):
                m[f'oh{k}'] = np.ascontiguousarray(
                    np.concatenate([oh[:, k * 2048:(k + 1) * 2048],
                                    oh[:, NP5 + k * 2048:NP5 + (k + 1) * 2048]], 1))
                m[f'bx{k}'] = np.ascontiguousarray(bx5[:, k * 2048:(k + 1) * 2048])
            in_maps.append(m)

    nc = _get_bass()
    global _LAST_IN_MAPS
    _LAST_IN_MAPS = in_maps
    res = run_bass_kernel_spmd(nc, in_maps, core_ids=list(range(8)))
    results = res.results

    out = np.zeros((NTREE, NGEN), np.float64)
    lnscale = 2.0 * np.log(SCALE)
    for t in range(NTREE):
        base = t * NT
        # level-5 log-nus from exact device f32 nus, logs in f64 on host
        # (fp8 scale correction is exact: nu is scaled by SCALE^2)
        for q in range(NQ):
            r = results[t * NQ + q]
            out[t] += np.log(r['nu5'].astype(np.float64)).sum(1) - NP5 * lnscale

        # leaf log-nus: histogram x log-table, exact in f64
        xs = x[base + STARTS[6]: base + STARTS[6] + K ** DEPTH]
        idx = (np.arange(K ** DEPTH) % K) * MSYM + xs
        counts = np.bincount(idx, minlength=K * MSYM).astype(np.float64)
        out[t] += counts @ llLeaf.reshape(K * MSYM, NGEN)

        # levels 4..1 + root on host from device tb4, f64 softmax math
        tb4 = np.concatenate(
            [results[t * NQ + q][f'tb4_{h}'].astype(np.float64)
             for q in range(NQ) for h in range(4)],
            axis=1)                                             # [128, 4096]
        tb = tb4.T.reshape(K ** 4, C, NGEN)                     # (u, i, g)
        for d in range(4, -1, -1):
            n_d = K ** d
            s_d = base + STARTS[d]
            x_d = x[s_d: s_d + n_d]
            bl = tb * np.transpose(smB[:, x_d], (1, 0, 2))      # (u, C, G)
            nu = bl.sum(1)
            out[t] += np.log(nu).sum(0)
            if d == 0:
                break
            beta = bl / nu[:, None]
            bch = beta.reshape(n_d // K, K, C, NGEN)
            tb = np.einsum('uljg,lijg->uig', bch, Mmat)
    return out.astype(np.float32)


# revision 7
# speedup vs baseline: 12.3320x; 12.3320x over previous
"""Bottom-Up Hidden Tree Markov Model upward pass on 8 Trainium2 NeuronCores.

Problem: complete 8-ary forest (2 trees x 299593 nodes, depth 6), C=8 hidden
states, 32 symbols, 16 independent generative models. Output: per-tree
log-likelihood (2, 16).

Sharding: core = (tree, quarter-of-tree). Each core runs the upward pass over
its quarter's two big levels (98.4% of all nodes): 65536 leaves -> 8192
level-5 betas (one fp8 DoubleRow matmul per 512 parents against the collapsed
leaf table T6, contracting all 256 (position, symbol) one-hot rows at once)
and 8192 -> 1024 level-4 t_betas (block-diagonal W matmuls). The host applies
the exact softmax recursion to the tiny tree top (levels 4..1 + root, 1.6% of
nodes) and the leaf log-nu histogram, both in float64.

Device pipeline per 512-parent chunk (engines balanced, dispatch-minimal):
  PE:   tb   = [T6a;T6b]^T [ohA;ohB]           (fp8 DoubleRow, PSUM f32)
  DVE:  bl   = tb * bx                          (bf16 out)
  PE:   nu_b = SR^T bl                          (per-g sums broadcast to all
                                                 128 partitions in one matmul)
  Act:  Ln(nu) with free-dim accumulation       -> per-chunk log-nu partials
  Pool: beta = bl / nu_b                        (divide, bf16 out)
Partition packing everywhere: p = i*16 + g  (i = hidden state, g = generator).

T6 and bx are carried in fp8_e4m3 scaled by 128 (one-hots exact, ~2^-4
relative quantization); the uniform 128^2 scale cancels in beta and is
subtracted from the accumulated log-nus on the host.
"""
import sys

import numpy as np

if '/opt/trn_rl_repo' not in sys.path:
    sys.path.insert(0, '/opt/trn_rl_repo')

import ml_dtypes

BF16 = ml_dtypes.bfloat16
F8 = ml_dtypes.float8_e4m3

K, DEPTH, NTREE, C, MSYM, NGEN = 8, 6, 2, 8, 32, 16
STARTS = [(K ** d - 1) // (K - 1) for d in range(DEPTH + 2)]
NT = STARTS[DEPTH + 1]          # 299593 nodes per tree
CG = C * NGEN                   # 128
NQ = 4                          # quarters per tree (core = tree x quarter)
LEAVES_Q = (K ** DEPTH) // NQ   # 65536 leaves per core
NP5 = LEAVES_Q // K             # 8192 level-5 parents per core
NP4 = NP5 // K                  # 1024 level-4 parents per core
CW = 512                        # chunk width (one PSUM bank of f32)
NCH = NP5 // CW                 # 16 chunks
SCALE = 128.0                   # fp8 range centering; cancels in beta
_SR_OFF = 256                   # byte offsets in the small packed table
TABB = 512


def _softmax64(x, axis):
    x = np.asarray(x, np.float64)
    e = np.exp(x - x.max(axis=axis, keepdims=True))
    return e / e.sum(axis=axis, keepdims=True)


def _build_tables(A, B, Pi, SP):
    """Small O(params) tables, f64 on host."""
    smA = _softmax64(A, 0)            # (C,C,K,G) over parent state i
    smB = _softmax64(B, 1)            # (C,M,G) over symbols
    smPi = _softmax64(Pi, 0)          # (C,K,G)
    smSP = _softmax64(SP, 0)          # (K,G)
    Mmat = smSP[:, None, None, :] * np.transpose(smA, (2, 0, 1, 3))  # [l,i,j,g]
    pb = smPi[:, :, None, :] * smB[:, None, :, :]      # (j, l, s, g)
    nuL = pb.sum(0)                                     # (l, s, g)
    betaLeaf = pb / nuL[None]
    llLeaf = np.log(nuL)                                # (l, s, g)
    T6 = np.einsum('lijg,jlsg->lsig', Mmat, betaLeaf)   # (l,s,i,g)
    T6f = (T6 * SCALE).reshape(K * MSYM, CG)            # rows (l,s), cols (i,g)
    Wl = np.zeros((K, CG, CG))
    ii = np.arange(C)
    for l in range(K):
        for g in range(NGEN):
            Wl[l, ii[:, None] * NGEN + g, ii[None, :] * NGEN + g] = Mmat[l, :, :, g].T
    Wt = np.concatenate([Wl[l] for l in range(K)], axis=1)   # [128, 1024]
    p = np.arange(CG)
    SR = (p[:, None] % NGEN == p[None, :] % NGEN).astype(np.float64)  # [128,128]
    BTcg = np.transpose(smB, (1, 0, 2)).reshape(MSYM, CG).T  # [(i,g), s]

    tabs = np.zeros((CG, TABB), np.uint8)
    tabs[:, 0:128] = T6f[:128].T.astype(F8).view(np.uint8)
    tabs[:, 128:256] = T6f[128:].T.astype(F8).view(np.uint8)
    tabs[:, _SR_OFF:_SR_OFF + 256] = SR.astype(BF16).view(np.uint8)
    return tabs, Wt.astype(BF16), Mmat, smB, BTcg, llLeaf


def _build_bass(repeat=1):
    import concourse.bass as bass
    import concourse.bacc as bacc
    import concourse.mybir as mybir
    from concourse import tile

    f32 = mybir.dt.float32
    bf16 = mybir.dt.bfloat16
    f8 = mybir.dt.float8e4
    u8 = mybir.dt.uint8
    Alu = mybir.AluOpType
    Act = mybir.ActivationFunctionType
    DR = mybir.MatmulPerfMode.DoubleRow

    nc = bacc.Bacc(None, target_bir_lowering=False)

    tabs_d = nc.dram_tensor('tabs', [CG, TABB], u8, kind='ExternalInput')
    wt_d = nc.dram_tensor('wt', [CG, 1024], bf16, kind='ExternalInput')
    oh_d = [nc.dram_tensor(f'oh{k}', [CG, 4096], f8, kind='ExternalInput')
            for k in range(4)]
    bx_d = [nc.dram_tensor(f'bx{k}', [CG, 2048], bf16, kind='ExternalInput')
            for k in range(4)]
    tb4_d = [nc.dram_tensor(f'tb4_{h}', [CG, 256], f32, kind='ExternalOutput')
             for h in range(4)]
    r5_d = nc.dram_tensor('r5', [NGEN, NP5], bf16, kind='ExternalOutput')

    with tile.TileContext(nc) as tc:
        with (
            tc.tile_pool(name='const', bufs=2) as constp,
            tc.tile_pool(name='oh', bufs=2) as ohp,
            tc.tile_pool(name='bx', bufs=2) as bxp,
            tc.tile_pool(name='bl', bufs=6) as blp,
            tc.tile_pool(name='beta', bufs=2) as betap,
            tc.tile_pool(name='acc', bufs=2) as accp,
            tc.tile_pool(name='ps_tb', bufs=3, space='PSUM') as ps_tb,
            tc.tile_pool(name='ps_nu', bufs=3, space='PSUM') as ps_nu,
            tc.tile_pool(name='ps_w', bufs=1, space='PSUM') as ps_w,
        ):
            for rep in range(repeat):
                r = f'_{rep}' if repeat > 1 else ''
                tabs_t = constp.tile([CG, TABB], u8, name=f'tabs{r}', tag='tabs')
                wt_t = constp.tile([CG, 1024], bf16, name=f'wt{r}', tag='wt')
                oh_t = [ohp.tile([CG, 4096], f8, name=f'oh{k}{r}', tag=f'oh{k}')
                        for k in range(4)]
                bx_t = bxp.tile([CG, NP5], bf16, name=f'bx{r}', tag='bx')
                beta5 = betap.tile([CG, NP5], bf16, name=f'b5{r}', tag='b5')
                r_all = betap.tile([CG, NP5], bf16, name=f'rall{r}', tag='rall')
                tb4_sb = accp.tile([CG, NP4], f32, name=f'tb4{r}', tag='tb4')

                # input DMAs in first-need order; Pool issues via SWDGE which
                # bypasses the serial HWDGE dispatch stage, so early tensors
                # go there to cut the startup trickle
                nc.sync.dma_start(tabs_t[:], tabs_d[:])
                nc.scalar.dma_start(oh_t[0][:], oh_d[0][:])
                nc.gpsimd.dma_start(bx_t[:, 0:2048], bx_d[0][:])
                nc.gpsimd.dma_start(oh_t[1][:], oh_d[1][:])
                nc.scalar.dma_start(bx_t[:, 2048:4096], bx_d[1][:])
                nc.sync.dma_start(oh_t[2][:], oh_d[2][:])
                nc.gpsimd.dma_start(bx_t[:, 4096:6144], bx_d[2][:])
                nc.sync.dma_start(oh_t[3][:], oh_d[3][:])
                nc.scalar.dma_start(bx_t[:, 6144:NP5], bx_d[3][:])
                nc.sync.dma_start(wt_t[:], wt_d[:])

                # fp8 DoubleRow weights: k-tile 0 = T6a, k-tile 1 = T6b
                t6ab = tabs_t[:, 0:256].bitcast(f8).rearrange(
                    'p (k q) -> p k q', k=2)
                SR = tabs_t[:, _SR_OFF:_SR_OFF + 256].bitcast(bf16)

                bview = beta5[:].rearrange('p (u l) -> p u l', l=K)
                bl_tiles = {}

                def emit_head(c):
                    k, j = c // 4, c % 4
                    tb_ps = ps_tb.tile([CG, CW], f32, name=f'tb{r}', tag='tb')
                    t = oh_t[k][:]
                    rhs = bass.AP(t.tensor, t.offset + j * CW,
                                  [t.ap[0], [2048, 2], [1, CW]])
                    nc.tensor.matmul(tb_ps[:], t6ab, rhs, start=True, stop=True,
                                     perf_mode=DR)
                    tb_sb = blp.tile([CG, CW], bf16, name=f'tbs{r}', tag='tbs')
                    nc.scalar.copy(tb_sb[:], tb_ps[:])
                    bl_t = blp.tile([CG, CW], bf16, name=f'bl{r}', tag='bl')
                    bl_eng = nc.vector if c % 8 < 5 else nc.gpsimd
                    bl_eng.tensor_mul(bl_t[:], tb_sb[:],
                                      bx_t[:, c * CW:(c + 1) * CW])
                    bl_tiles[c] = bl_t

                def emit_tail(c):
                    bl_t = bl_tiles.pop(c)
                    nu_ps = ps_nu.tile([CG, CW], f32, name=f'nu{r}', tag='nu')
                    nc.tensor.matmul(nu_ps[:], SR, bl_t[:], start=True, stop=True)

                    # GPSIMD/Pool cannot touch PSUM and DVE has no divide ISA
                    # op, so renorm = DVE reciprocal (PSUM -> SBUF bf16, the
                    # SR matmul already broadcast nu to all 128 partitions)
                    # followed by an all-SBUF multiply on Pool
                    r_sl = r_all[:, c * CW:(c + 1) * CW]
                    with nc.allow_low_precision(reason='bf16 recip, validated vs f64 host'):
                        nc.vector.reciprocal(r_sl, nu_ps[:])
                    mul_eng = nc.vector if (c % 8 < 5 or c == NCH - 1) else nc.gpsimd
                    mul_eng.tensor_mul(beta5[:, c * CW:(c + 1) * CW],
                                       bl_t[:], r_sl)

                def emit_w(ph):
                    # level 5 -> 4 over parents [128*ph, 128*(ph+1)), i.e.
                    # beta5 chunks 2ph..2ph+1; drain on DVE; ship per quarter
                    w_ps = ps_w.tile([CG, 128], f32, name=f'w{ph}{r}',
                                     tag=f'w{ph % 2}')
                    for l in range(K):
                        nc.tensor.matmul(w_ps[:], wt_t[:, 128 * l:128 * (l + 1)],
                                         bview[:, ph * 128:(ph + 1) * 128, l],
                                         start=(l == 0), stop=(l == K - 1))
                    nc.scalar.copy(tb4_sb[:, ph * 128:(ph + 1) * 128], w_ps[:])
                    if ph % 2 == 1:
                        qd = ph // 2
                        nc.scalar.dma_start(
                            tb4_d[qd][:], tb4_sb[:, qd * 256:(qd + 1) * 256])

                # software-pipelined level 6 -> 5 (tail lags head by LAG);
                # W phase ph consumes chunks 2ph..2ph+1
                LAG = 2
                for c in range(NCH):
                    emit_head(c)
                    if c >= LAG:
                        emit_tail(c - LAG)
                        if (c - LAG) % 2 == 1:
                            emit_w((c - LAG) // 2)
                for c in range(NCH - LAG, NCH):
                    emit_tail(c)
                    if c % 2 == 1:
                        emit_w(c // 2)

                nc.sync.dma_start(r5_d[:], r_all[0:NGEN, :])
    if not nc.is_finalized():
        nc.finalize()
    return nc


_BASS_CACHE = {}


def _get_bass(repeat=1):
    if repeat not in _BASS_CACHE:
        _BASS_CACHE[repeat] = _build_bass(repeat)
    return _BASS_CACHE[repeat]


def kernel(**inputs):
    from concourse.bass_utils import run_bass_kernel_spmd

    A = np.asarray(inputs['A']); B = np.asarray(inputs['B'])
    Pi = np.asarray(inputs['Pi']); SP = np.asarray(inputs['SP'])
    x = np.asarray(inputs['x'])

    tabs, Wt_bf, Mmat, smB, BTcg, llLeaf = _build_tables(A, B, Pi, SP)
    BT_bf = BTcg.astype(BF16)

    in_maps = []
    for t in range(NTREE):
        base = t * NT
        for q in range(NQ):
            s6 = base + STARTS[6] + q * LEAVES_Q
            xs_t = x[s6: s6 + LEAVES_Q].reshape(NP5, K).T      # [8, 8192]
            s5 = base + STARTS[5] + q * NP5
            x5 = x[s5: s5 + NP5]
            oh = np.zeros((CG, 2 * NP5), F8)
            cols = np.arange(NP5)
            one = F8(1.0)
            for l in range(4):
                oh[l * MSYM + xs_t[l], cols] = one
                oh[l * MSYM + xs_t[4 + l], NP5 + cols] = one
            bx5 = BT_bf[:, x5]                                  # [128, 8192]
            m = {'tabs': tabs, 'wt': Wt_bf}
            for k in range(4):
                m[f'oh{k}'] = np.ascontiguousarray(
                    np.concatenate([oh[:, k * 2048:(k + 1) * 2048],
                                    oh[:, NP5 + k * 2048:NP5 + (k + 1) * 2048]], 1))
                m[f'bx{k}'] = np.ascontiguousarray(bx5[:, k * 2048:(k + 1) * 2048])
            in_maps.append(m)

    nc = _get_bass()
    global _LAST_IN_MAPS
    _LAST_IN_MAPS = in_maps
    res = run_bass_kernel_spmd(nc, in_maps, core_ids=list(range(8)))
    results = res.results

    out = np.zeros((NTREE, NGEN), np.float64)
    lnscale = np.log(SCALE)   # only T6 is fp8-scaled now; bx is bf16 unscaled
    for t in range(NTREE):
        base = t * NT
        # level-5 log-nus from device reciprocals (ln nu = -ln r), f64 on
        # host; the fp8 scale correction is exact (nu is scaled by SCALE^2)
        for q in range(NQ):
            r = results[t * NQ + q]
            out[t] += -np.log(r['r5'].astype(np.float64)).sum(1) - NP5 * lnscale

        # leaf log-nus: histogram x log-table, exact in f64
        xs = x[base + STARTS[6]: base + STARTS[6] + K ** DEPTH]
        idx = (np.arange(K ** DEPTH) % K) * MSYM + xs
        counts = np.bincount(idx, minlength=K * MSYM).astype(np.float64)
        out[t] += counts @ llLeaf.reshape(K * MSYM, NGEN)

        # levels 4..1 + root on host from device tb4, f64 softmax math
        tb4 = np.concatenate(
            [results[t * NQ + q][f'tb4_{h}'].astype(np.float64)
             for q in range(NQ) for h in range(4)],
            axis=1)                                             # [128, 4096]
        tb = tb4.T.reshape(K ** 4, C, NGEN)                     # (u, i, g)
        for d in range(4, -1, -1):
            n_d = K ** d
            s_d = base + STARTS[d]
            x_d = x[s_d: s_d + n_d]
            bl = tb * np.transpose(smB[:, x_d], (1, 0, 2))      # (u, C, G)
            nu = bl.sum(1)
            out[t] += np.log(nu).sum(0)
            if d == 0:
                break
            beta = bl / nu[:, None]
            bch = beta.reshape(n_d // K, K, C, NGEN)
            tb = np.einsum('uljg,lijg->uig', bch, Mmat)
    return out.astype(np.float32)
